# revision 19
# baseline (speedup 1.0000x reference)
"""Embedding lookup (gather) kernel for Trainium2, 8 NeuronCores.

Problem: out[i] = table[value_tensors[i]] for 212992 indices into a
[1M, 128] f32 table, reshaped to [8192, 26, 128]. (row_offsets is
arange, so the CSR segment-sum is the identity; a host-side fallback
handles the general case.)

Sharding: model-parallel by table row (range partition); core c owns
rows [c*125000, (c+1)*125000), uploaded as an fp16 copy (32MB). The
host routes lookups to cores, each core gathers its rows on-device
with SWDGE dma_gather, and the host scatters rows back to original
positions (HugeCTR's localized-embedding all-to-all, done at unshard
time). fp16 halves HBM traffic; the rel-err contract (2e-2) dwarfs
fp16 rounding (~5e-4).

The gather is Q7-ucode descriptor-generation bound (~7.4ns/idx per
SWDGE queue, 4 queues), so the kernel minimizes descriptor count:
  - dedupe: only unique rows are gathered (~10% of lookups repeat);
  - pair-merge: consecutive unique rows (r, r+1) become ONE 512B
    descriptor via an overlapping-window source AP (elem=256 f16,
    elem_step=128) — ~16% fewer descriptors;
  - exact per-chunk counts are loaded from SBUF into registers
    (reg_load) so padding generates no descriptors; idx arrays are
    padded with -1 (the ucode requires num_idxs_reg == count of
    non-negative indices in the chunk — a mismatch faults the device).
Chunks are issued in strict queue rotation (s+j)%4 — consecutive
instructions to the same queue head-of-line block the Pool engine.
Each chunk's rows are written out on completion, alternating the two
HWDGE rings (Sync/Scalar); the final round's chunks are small so the
post-gather write tail is short.

Per-core timeline (~77us): ~17us fixed Q7 library reload (idx/cnt
DMAs and register loads hide under it), ~47us descriptor generation
(round-0 pipeline fill + 42us/queue of ucode), ~12us transfer drain +
final writes (the 16 DMA engines run ~48us of descriptor execution,
co-binding with generation).
"""

import numpy as np

VOCAB = 1_000_000
BATCH = 8192
SLOTS = 26
VEC = 128
NCORES = 8
NSUB = 4  # range bins per core; int16 gather idx needs rows <= 32767
RSUB = VOCAB // (NCORES * NSUB)  # 31250 rows per bin
SHARD = RSUB * NSUB  # 125000 rows per core
P = 128

LAST_RUN = None  # BassKernelResults of the most recent device run (for test.py)

# Exact per-chunk gather counts via reg_load registers (skips pad
# descriptors, ~4us). Falls back to compile-time padded counts if False.
EXACT_REGS = True


def _chunk_plan(NP_, NS_):
    """Per-bin chunk plan: list of (cls, offset, size), cls 'p'|'s'.
    Round order = list order. Small chunk first (the first instruction
    after the Q7 library reload holds the Pool engine for its whole
    ucode run, so make it cheap), small chunk last (short write tail);
    sizes are multiples of 128."""
    head = 256 if NS_ > 3072 else 0
    tail = 512 if NS_ - head > 2560 else 0
    plan = []
    so = 0
    if head:
        plan.append(("s", 0, head))
        so = head
    plan.append(("p", 0, NP_))
    rem = NS_ - head - tail
    while rem > 0:
        c = min(2048, rem)
        plan.append(("s", so, c))
        so += c
        rem -= c
    if tail:
        plan.append(("s", so, tail))
    return plan


def _build_program(NP_, NS_, plan, const_full=None):
    """One SPMD program for all 8 cores.

    Per core:
      shard16 [SHARD, VEC] f16    - this core's 4 bins, fp16
      idxp    [P, NSUB*NP_/16] i16 - pair-start local idx, wrapped
      idxs    [P, NSUB*NS_/16] i16 - single local idx, wrapped
      cnt     [1, NSUB*len(plan)] i32 - exact per-chunk counts
      out_p   [P, NSUB*(NP_/128)*2*VEC] f16 - gathered pair rows
      out_s   [P, NSUB*(NS_/128)*VEC] f16   - gathered single rows
    """
    import concourse.bacc as bacc
    from concourse import mybir
    from concourse.ap import AP
    from concourse.library_config import mlp

    SP_, SS_ = NP_ // 16, NS_ // 16
    CP_, CS_ = NP_ // 128, NS_ // 128
    ncls = len(plan)

    nc = bacc.Bacc("TRN2", num_swdge_queues=4)
    shard16 = nc.declare_dram_parameter(
        "shard16", [SHARD, VEC], mybir.dt.float16, isOutput=False
    )
    idxp = nc.declare_dram_parameter("idxp", [P, NSUB * SP_], mybir.dt.int16, isOutput=False)
    idxs = nc.declare_dram_parameter("idxs", [P, NSUB * SS_], mybir.dt.int16, isOutput=False)
    cnt = nc.declare_dram_parameter("cnt", [1, NSUB * ncls], mybir.dt.int32, isOutput=False)
    out_p = nc.declare_dram_parameter(
        "out_p", [P, NSUB * CP_ * 2 * VEC], mybir.dt.float16, isOutput=True
    )
    out_s = nc.declare_dram_parameter(
        "out_s", [P, NSUB * CS_ * VEC], mybir.dt.float16, isOutput=True
    )

    sem_idx = nc.alloc_semaphore()
    sem_cnt = nc.alloc_semaphore()
    sem_out = nc.alloc_semaphore()

    idxp_sb = nc.alloc_sbuf_tensor("idxp_sb", [P, NSUB * SP_], mybir.dt.int16).ap()
    idxs_sb = nc.alloc_sbuf_tensor("idxs_sb", [P, NSUB * SS_], mybir.dt.int16).ap()
    cnt_sb = nc.alloc_sbuf_tensor("cnt_sb", [1, NSUB * ncls], mybir.dt.int32).ap()
    gp = nc.alloc_sbuf_tensor("gp", [P, NSUB, CP_, 2 * VEC], mybir.dt.float16).ap()
    gs = nc.alloc_sbuf_tensor("gs", [P, NSUB, CS_, VEC], mybir.dt.float16).ap()

    def pair_src(s):
        # overlapping windows: window r = rows (r, r+1) of the bin
        return AP(shard16, (s * RSUB) * VEC, [[VEC, RSUB - 1], [1, 2 * VEC]])

    nc.gpsimd.load_library(mlp)
    nc.sync.dma_start(out=cnt_sb[:, :], in_=cnt[:, :]).then_inc(sem_cnt, 16)
    nc.sync.dma_start(out=idxp_sb[:], in_=idxp[:, :]).then_inc(sem_idx, 16)
    nc.scalar.dma_start(out=idxs_sb[:], in_=idxs[:, :]).then_inc(sem_idx, 16)

    # Exact per-chunk counts -> registers (hidden under the ~16.5us Q7
    # library reload, as are the idx loads).
    nc.gpsimd.wait_ge(sem_cnt, 16)
    regs = {}
    if EXACT_REGS:
        # Chunks that are full on every core (middle singles chunks) use a
        # shared immediate register; only variable chunks pay a reg_load
        # (reg_loads run post-library-reload and delay round 0).
        const_regs = {}
        for s in range(NSUB):
            for j, (cls, o, sz) in enumerate(plan):
                if const_full and const_full[j]:
                    if sz not in const_regs:
                        const_regs[sz] = nc.gpsimd.to_reg(sz)
                    regs[(s, j)] = const_regs[sz]
                else:
                    r = nc.gpsimd.alloc_register(f"cnt{s}_{j}")
                    nc.gpsimd.reg_load(r, cnt_sb[:, s * ncls + j : s * ncls + j + 1])
                    regs[(s, j)] = r
    else:
        const_regs = {}
        for cls, o, sz in plan:
            if sz not in const_regs:
                const_regs[sz] = nc.gpsimd.to_reg(sz)
        for s in range(NSUB):
            for j, (cls, o, sz) in enumerate(plan):
                regs[(s, j)] = const_regs[sz]
    nc.gpsimd.wait_ge(sem_idx, 32)

    out_p_v = out_p.rearrange("p (s c v) -> p s c v", s=NSUB, c=CP_, v=2 * VEC)
    out_s_v = out_s.rearrange("p (s c v) -> p s c v", s=NSUB, c=CS_, v=VEC)
    sem_g = {}
    writes = []  # (j, out_region, sbuf_region): one strided write per round
    for j, (cls, o, sz) in enumerate(plan):
        if cls == "p":
            writes.append(
                (j,
                 out_p_v[:, :, o // 128 : (o + sz) // 128, :],
                 gp[:, :, o // 128 : (o + sz) // 128, :])
            )
        else:
            writes.append(
                (j,
                 out_s_v[:, :, o // 128 : (o + sz) // 128, :],
                 gs[:, :, o // 128 : (o + sz) // 128, :])
            )
        for s in range(NSUB):
            q = (s + j) % 4
            sem = nc.alloc_semaphore(f"g{s}_{j}")
            sem_g[(s, j)] = sem
            if cls == "p":
                nc.gpsimd.dma_gather(
                    gp[:, s, o // 128 : (o + sz) // 128, :],
                    pair_src(s),
                    idxp_sb[:, s * SP_ + o // 16 : s * SP_ + (o + sz) // 16],
                    sz, regs[(s, j)], 2 * VEC, elem_step=VEC,
                    queue_num=q, single_packet=False,
                ).then_inc(sem, 16)
            else:
                nc.gpsimd.dma_gather(
                    gs[:, s, o // 128 : (o + sz) // 128, :],
                    shard16[s * RSUB : (s + 1) * RSUB, :],
                    idxs_sb[:, s * SS_ + o // 16 : s * SS_ + (o + sz) // 16],
                    sz, regs[(s, j)], VEC,
                    queue_num=q, single_packet=False,
                ).then_inc(sem, 16)

    wengs = [nc.sync, nc.scalar]
    for wi, (j, dst, src) in enumerate(writes):
        eng = wengs[wi % 2]
        for s in range(NSUB):
            eng.wait_ge(sem_g[(s, j)], 16)
        eng.dma_start(out=dst, in_=src).then_inc(sem_out, 16)
    nc.sync.wait_ge(sem_out, 16 * len(writes))
    nc.finalize()
    return nc


def _gather_on_device(table, v):
    """emb[i] = table[v[i]] computed on 8 NeuronCores (fp16 payload)."""
    global LAST_RUN
    from concourse.bass_utils import run_bass_kernel_spmd

    total = v.shape[0]
    table16 = table.astype(np.float16)

    # Per core: sort+dedupe, split unique rows into consecutive-pair
    # starts and singles per bin, and record the inverse mapping.
    per_core = []  # (pos, inv_u, pair_lists, single_lists, maps)
    for c in range(NCORES):
        lo, hi = c * SHARD, (c + 1) * SHARD
        mask = (v >= lo) & (v < hi)
        pos = np.nonzero(mask)[0]
        u, inv = np.unique(v[pos] - lo, return_inverse=True)
        b_u = (u // RSUB).astype(np.int32)
        pair_lists, single_lists, maps = [], [], []
        for s in range(NSUB):
            us = u[b_u == s] - s * RSUB
            n = len(us)
            if n == 0:
                pair_lists.append(np.zeros(0, np.int16))
                single_lists.append(np.zeros(0, np.int16))
                maps.append((np.zeros(0, bool), np.zeros(0, np.int64), np.zeros(0, np.int64)))
                continue
            brk = np.empty(n, bool)
            brk[0] = True
            brk[1:] = np.diff(us) != 1
            run_id = np.cumsum(brk) - 1
            run_start = np.nonzero(brk)[0]
            run_len = np.diff(np.append(run_start, n))
            r = np.arange(n) - run_start[run_id]
            is_pair = r < 2 * (run_len[run_id] // 2)
            pair_base = np.concatenate(([0], np.cumsum(run_len // 2)))[:-1]
            single_base = np.concatenate(([0], np.cumsum(run_len % 2)))[:-1]
            pair_slot = pair_base[run_id] + r // 2  # valid where is_pair
            single_slot = single_base[run_id]  # valid where ~is_pair
            sub = r % 2
            pair_starts = us[is_pair & (sub == 0)]
            singles = us[~is_pair]
            pair_lists.append(pair_starts.astype(np.int16))
            single_lists.append(singles.astype(np.int16))
            maps.append((is_pair, np.where(is_pair, pair_slot * 2 + sub, 0), single_slot))
        per_core.append((pos, inv, pair_lists, single_lists, maps))

    NP_ = max(128, ((max(len(p) for pc in per_core for p in pc[2]) + 127) // 128) * 128)
    NS_ = max(128, ((max(len(s) for pc in per_core for s in pc[3]) + 127) // 128) * 128)
    plan = _chunk_plan(NP_, NS_)
    ncls = len(plan)
    SP_, SS_ = NP_ // 16, NS_ // 16
    CP_, CS_ = NP_ // 128, NS_ // 128

    def wrap(li):
        # idx layout the gather ucode expects: wrapped over 16 partitions,
        # replicated to all 8 partition groups
        N = len(li)
        w = np.zeros((16, N // 16), np.int16)
        ar = np.arange(N)
        w[ar % 16, ar // 16] = li
        return np.tile(w, (8, 1))

    in_maps = []
    for c in range(NCORES):
        _, _, pair_lists, single_lists, _ = per_core[c]
        # The ucode contract: num_idxs_reg == count of NON-NEGATIVE idx in
        # the chunk's slice (violations fault the exec unit). Real entries,
        # then 0-pads up to each chunk's register count, then -1.
        cnts = np.empty((NSUB, ncls), np.int32)
        lips, liss = [], []
        for s in range(NSUB):
            lip = np.full(NP_, -1, np.int16)
            lip[: len(pair_lists[s])] = pair_lists[s]
            lis_ = np.full(NS_, -1, np.int16)
            lis_[: len(single_lists[s])] = single_lists[s]
            for j, (cls, o, sz) in enumerate(plan):
                li = lip if cls == "p" else lis_
                n = len(pair_lists[s]) if cls == "p" else len(single_lists[s])
                e = max(min(n - o, sz), min(16, sz))
                lo, hi = max(n, o), o + e
                if hi > lo:
                    li[lo:hi] = 0
                cnts[s, j] = e
            lips.append(lip)
            liss.append(lis_)
        in_maps.append({
            "shard16": np.ascontiguousarray(table16[c * SHARD : (c + 1) * SHARD]),
            "idxp": np.ascontiguousarray(np.concatenate([wrap(x) for x in lips], axis=1)),
            "idxs": np.ascontiguousarray(np.concatenate([wrap(x) for x in liss], axis=1)),
            "cnt": cnts.reshape(1, NSUB * ncls),
        })

    const_full = [
        all(
            min(len(pc[2][s] if cls == "p" else pc[3][s]) - o, sz) == sz
            for pc in per_core
            for s in range(NSUB)
        )
        for (cls, o, sz) in plan
    ]
    nc = _build_program(NP_, NS_, plan, const_full)
    LAST_RUN = run_bass_kernel_spmd(nc, in_maps, list(range(NCORES)))
    res = LAST_RUN.results

    emb = np.empty((total, VEC), np.float16)
    for c in range(NCORES):
        pos, inv, pair_lists, single_lists, maps = per_core[c]
        op = np.asarray(res[c]["out_p"]).view(np.float16).reshape(P, NSUB, CP_, 2 * VEC)
        os_ = np.asarray(res[c]["out_s"]).view(np.float16).reshape(P, NSUB, CS_, VEC)
        emb_u_parts = []
        for s in range(NSUB):
            is_pair, pair_row, single_slot = maps[s]
            n = len(is_pair)
            if n == 0:
                continue
            # pair slot k landed at [k%128, k//128, :] as 2*VEC elems
            prows = op[:, s].transpose(1, 0, 2).reshape(NP_ * 2, VEC)
            srows = os_[:, s].transpose(1, 0, 2).reshape(NS_, VEC)
            eu = np.empty((n, VEC), np.float16)
            eu[is_pair] = prows[pair_row[is_pair]]
            eu[~is_pair] = srows[single_slot[~is_pair]]
            emb_u_parts.append(eu)
        emb_u = np.concatenate(emb_u_parts, axis=0)
        emb[pos] = emb_u[inv]
    return emb.astype(np.float32)


def kernel(table, row_offsets, value_tensors, nnz_array=None, output_shape=None):
    table = np.ascontiguousarray(np.asarray(table, dtype=np.float32))
    assert table.shape == (VOCAB, VEC)
    v = np.asarray(value_tensors).astype(np.int64).ravel()
    total = v.shape[0]

    emb = _gather_on_device(table, v)

    n_rows = BATCH * SLOTS
    ro = np.asarray(row_offsets).astype(np.int64).ravel()
    if total == n_rows and np.array_equal(ro, np.arange(total + 1)):
        return emb.reshape(BATCH, SLOTS, VEC)
    # General CSR fallback (never hit with the reference's arange offsets):
    # sum-combine values per segment on the host.
    seg = np.searchsorted(ro, np.arange(total), side="right") - 1
    combined = np.zeros((n_rows, VEC), np.float32)
    np.add.at(combined, seg, emb)
    return combined.reshape(BATCH, SLOTS, VEC)


# revision 20
# speedup vs baseline: 1.1873x; 1.1873x over previous
"""Embedding lookup (gather) kernel for Trainium2, 8 NeuronCores.

Problem: out[i] = table[value_tensors[i]] for 212992 indices into a
[1M, 128] f32 table, reshaped to [8192, 26, 128]. (row_offsets is
arange, so the CSR segment-sum is the identity; a host-side fallback
handles the general case.)

Sharding: model-parallel by table row (range partition); core c owns
rows [c*125000, (c+1)*125000), uploaded as an fp16 copy (32MB). The
host routes lookups to cores, each core gathers its rows on-device
with SWDGE dma_gather, and the host scatters rows back to original
positions (HugeCTR's localized-embedding all-to-all, done at unshard
time). fp16 halves HBM traffic; the rel-err contract (2e-2) dwarfs
fp16 rounding (~5e-4).

The gather is Q7-ucode descriptor-generation bound (~7.4ns/idx per
SWDGE queue, 4 queues), so the kernel minimizes descriptor count:
  - dedupe: only unique rows are gathered (~10% of lookups repeat);
  - pair-merge: consecutive unique rows (r, r+1) become ONE 512B
    descriptor via an overlapping-window source AP (elem=256 f16,
    elem_step=128) — ~16% fewer descriptors;
  - exact per-chunk counts are loaded from SBUF into registers
    (reg_load) so padding generates no descriptors; idx arrays are
    padded with -1 (the ucode requires num_idxs_reg == count of
    non-negative indices in the chunk — a mismatch faults the device).
Chunks are issued in strict queue rotation (s+j)%4 — consecutive
instructions to the same queue head-of-line block the Pool engine.
Each chunk's rows are written out on completion, alternating the two
HWDGE rings (Sync/Scalar); the final round's chunks are small so the
post-gather write tail is short.

Per-core timeline (~77us): ~17us fixed Q7 library reload (idx/cnt
DMAs and register loads hide under it), ~47us descriptor generation
(round-0 pipeline fill + 42us/queue of ucode), ~12us transfer drain +
final writes (the 16 DMA engines run ~48us of descriptor execution,
co-binding with generation).
"""

import numpy as np

VOCAB = 1_000_000
BATCH = 8192
SLOTS = 26
VEC = 128
NCORES = 8
NSUB = 4  # range bins per core; int16 gather idx needs rows <= 32767
RSUB = VOCAB // (NCORES * NSUB)  # 31250 rows per bin
SHARD = RSUB * NSUB  # 125000 rows per core
P = 128

LAST_RUN = None  # BassKernelResults of the most recent device run (for test.py)

# Exact per-chunk gather counts via reg_load registers (skips pad
# descriptors, ~4us). Falls back to compile-time padded counts if False.
EXACT_REGS = True


def _chunk_plan(NP_, NS_):
    """Per-bin chunk plan: list of (cls, offset, size), cls 'p'|'s'.
    Round order = list order. Small chunk first (the first instruction
    after the Q7 library reload holds the Pool engine for its whole
    ucode run, so make it cheap), small chunk last (short write tail);
    sizes are multiples of 128."""
    head = 256 if NS_ > 3072 else 0
    tail = 512 if NS_ - head > 2560 else 0
    plan = []
    so = 0
    if head:
        plan.append(("s", 0, head))
        so = head
    plan.append(("p", 0, NP_))
    rem = NS_ - head - tail
    while rem > 0:
        c = min(2048, rem)
        plan.append(("s", so, c))
        so += c
        rem -= c
    if tail:
        plan.append(("s", so, tail))
    return plan


def _build_program(NP_, NS_, plan, const_full=None):
    """One SPMD program for all 8 cores.

    Per core:
      shard16 [SHARD, VEC] f16    - this core's 4 bins, fp16
      idxp    [P, NSUB*NP_/16] i16 - pair-start local idx, wrapped
      idxs    [P, NSUB*NS_/16] i16 - single local idx, wrapped
      cnt     [1, NSUB*len(plan)] i32 - exact per-chunk counts
      out_p   [P, NSUB*(NP_/128)*2*VEC] f16 - gathered pair rows
      out_s   [P, NSUB*(NS_/128)*VEC] f16   - gathered single rows
    """
    import concourse.bacc as bacc
    from concourse import mybir
    from concourse.ap import AP
    from concourse.library_config import mlp

    SP_, SS_ = NP_ // 16, NS_ // 16
    CP_, CS_ = NP_ // 128, NS_ // 128
    ncls = len(plan)

    nc = bacc.Bacc("TRN2", num_swdge_queues=4)
    shard16 = nc.declare_dram_parameter(
        "shard16", [SHARD, VEC], mybir.dt.float16, isOutput=False
    )
    idxp = nc.declare_dram_parameter("idxp", [P, NSUB * SP_], mybir.dt.int16, isOutput=False)
    idxs = nc.declare_dram_parameter("idxs", [P, NSUB * SS_], mybir.dt.int16, isOutput=False)
    cnt = nc.declare_dram_parameter("cnt", [1, NSUB * ncls], mybir.dt.int32, isOutput=False)
    out_p = nc.declare_dram_parameter(
        "out_p", [P, NSUB * CP_ * 2 * VEC], mybir.dt.float16, isOutput=True
    )
    out_s = nc.declare_dram_parameter(
        "out_s", [P, NSUB * CS_ * VEC], mybir.dt.float16, isOutput=True
    )

    sem_idx = nc.alloc_semaphore()
    sem_cnt = nc.alloc_semaphore()
    sem_out = nc.alloc_semaphore()

    idxp_sb = nc.alloc_sbuf_tensor("idxp_sb", [P, NSUB * SP_], mybir.dt.int16).ap()
    idxs_sb = nc.alloc_sbuf_tensor("idxs_sb", [P, NSUB * SS_], mybir.dt.int16).ap()
    cnt_sb = nc.alloc_sbuf_tensor("cnt_sb", [1, NSUB * ncls], mybir.dt.int32).ap()
    gp = nc.alloc_sbuf_tensor("gp", [P, NSUB, CP_, 2 * VEC], mybir.dt.float16).ap()
    gs = nc.alloc_sbuf_tensor("gs", [P, NSUB, CS_, VEC], mybir.dt.float16).ap()

    def pair_src(s):
        # overlapping windows: window r = rows (r, r+1) of the bin
        return AP(shard16, (s * RSUB) * VEC, [[VEC, RSUB - 1], [1, 2 * VEC]])

    nc.gpsimd.load_library(mlp)
    nc.sync.dma_start(out=cnt_sb[:, :], in_=cnt[:, :]).then_inc(sem_cnt, 16)
    nc.sync.dma_start(out=idxp_sb[:], in_=idxp[:, :]).then_inc(sem_idx, 16)
    nc.scalar.dma_start(out=idxs_sb[:], in_=idxs[:, :]).then_inc(sem_idx, 16)

    # Exact per-chunk counts -> registers (hidden under the ~16.5us Q7
    # library reload, as are the idx loads).
    nc.gpsimd.wait_ge(sem_cnt, 16)
    regs = {}
    if EXACT_REGS:
        # Chunks that are full on every core (middle singles chunks) use a
        # shared immediate register; only variable chunks pay a reg_load
        # (reg_loads run post-library-reload and delay round 0).
        const_regs = {}
        for s in range(NSUB):
            for j, (cls, o, sz) in enumerate(plan):
                if const_full and const_full[j]:
                    if sz not in const_regs:
                        const_regs[sz] = nc.gpsimd.to_reg(sz)
                    regs[(s, j)] = const_regs[sz]
                else:
                    r = nc.gpsimd.alloc_register(f"cnt{s}_{j}")
                    nc.gpsimd.reg_load(r, cnt_sb[:, s * ncls + j : s * ncls + j + 1])
                    regs[(s, j)] = r
    else:
        const_regs = {}
        for cls, o, sz in plan:
            if sz not in const_regs:
                const_regs[sz] = nc.gpsimd.to_reg(sz)
        for s in range(NSUB):
            for j, (cls, o, sz) in enumerate(plan):
                regs[(s, j)] = const_regs[sz]
    nc.gpsimd.wait_ge(sem_idx, 32)

    sem_g = {}
    writes = []  # (s, j, out_region, sbuf_region) in issue order
    for j, (cls, o, sz) in enumerate(plan):
        for s in range(NSUB):
            q = (s + j) % 4
            sem = nc.alloc_semaphore(f"g{s}_{j}")
            sem_g[(s, j)] = sem
            if cls == "p":
                nc.gpsimd.dma_gather(
                    gp[:, s, o // 128 : (o + sz) // 128, :],
                    pair_src(s),
                    idxp_sb[:, s * SP_ + o // 16 : s * SP_ + (o + sz) // 16],
                    sz, regs[(s, j)], 2 * VEC, elem_step=VEC,
                    queue_num=q, single_packet=False,
                ).then_inc(sem, 16)
                writes.append(
                    (s, j,
                     out_p[:, (s * CP_ + o // 128) * 2 * VEC : (s * CP_ + (o + sz) // 128) * 2 * VEC],
                     gp[:, s, o // 128 : (o + sz) // 128, :].rearrange("p c v -> p (c v)"))
                )
            else:
                nc.gpsimd.dma_gather(
                    gs[:, s, o // 128 : (o + sz) // 128, :],
                    shard16[s * RSUB : (s + 1) * RSUB, :],
                    idxs_sb[:, s * SS_ + o // 16 : s * SS_ + (o + sz) // 16],
                    sz, regs[(s, j)], VEC,
                    queue_num=q, single_packet=False,
                ).then_inc(sem, 16)
                writes.append(
                    (s, j,
                     out_s[:, (s * CS_ + o // 128) * VEC : (s * CS_ + (o + sz) // 128) * VEC],
                     gs[:, s, o // 128 : (o + sz) // 128, :].rearrange("p c v -> p (c v)"))
                )

    wengs = [nc.sync, nc.scalar]
    for wi, (s, j, dst, src) in enumerate(writes):
        eng = wengs[wi % 2]
        eng.wait_ge(sem_g[(s, j)], 16)
        eng.dma_start(out=dst, in_=src).then_inc(sem_out, 16)
    nc.sync.wait_ge(sem_out, 16 * len(writes))
    nc.finalize()
    return nc


def _gather_on_device(table, v):
    """emb[i] = table[v[i]] computed on 8 NeuronCores (fp16 payload)."""
    global LAST_RUN
    from concourse.bass_utils import run_bass_kernel_spmd

    total = v.shape[0]
    table16 = table.astype(np.float16)

    # Per core: sort+dedupe, split unique rows into consecutive-pair
    # starts and singles per bin, and record the inverse mapping.
    per_core = []  # (pos, inv_u, pair_lists, single_lists, maps)
    for c in range(NCORES):
        lo, hi = c * SHARD, (c + 1) * SHARD
        mask = (v >= lo) & (v < hi)
        pos = np.nonzero(mask)[0]
        u, inv = np.unique(v[pos] - lo, return_inverse=True)
        b_u = (u // RSUB).astype(np.int32)
        pair_lists, single_lists, maps = [], [], []
        for s in range(NSUB):
            us = u[b_u == s] - s * RSUB
            n = len(us)
            if n == 0:
                pair_lists.append(np.zeros(0, np.int16))
                single_lists.append(np.zeros(0, np.int16))
                maps.append((np.zeros(0, bool), np.zeros(0, np.int64), np.zeros(0, np.int64)))
                continue
            brk = np.empty(n, bool)
            brk[0] = True
            brk[1:] = np.diff(us) != 1
            run_id = np.cumsum(brk) - 1
            run_start = np.nonzero(brk)[0]
            run_len = np.diff(np.append(run_start, n))
            r = np.arange(n) - run_start[run_id]
            is_pair = r < 2 * (run_len[run_id] // 2)
            pair_base = np.concatenate(([0], np.cumsum(run_len // 2)))[:-1]
            single_base = np.concatenate(([0], np.cumsum(run_len % 2)))[:-1]
            pair_slot = pair_base[run_id] + r // 2  # valid where is_pair
            single_slot = single_base[run_id]  # valid where ~is_pair
            sub = r % 2
            pair_starts = us[is_pair & (sub == 0)]
            singles = us[~is_pair]
            pair_lists.append(pair_starts.astype(np.int16))
            single_lists.append(singles.astype(np.int16))
            maps.append((is_pair, np.where(is_pair, pair_slot * 2 + sub, 0), single_slot))
        per_core.append((pos, inv, pair_lists, single_lists, maps))

    NP_ = max(128, ((max(len(p) for pc in per_core for p in pc[2]) + 127) // 128) * 128)
    NS_ = max(128, ((max(len(s) for pc in per_core for s in pc[3]) + 127) // 128) * 128)
    plan = _chunk_plan(NP_, NS_)
    ncls = len(plan)
    SP_, SS_ = NP_ // 16, NS_ // 16
    CP_, CS_ = NP_ // 128, NS_ // 128

    def wrap(li):
        # idx layout the gather ucode expects: wrapped over 16 partitions,
        # replicated to all 8 partition groups
        N = len(li)
        w = np.zeros((16, N // 16), np.int16)
        ar = np.arange(N)
        w[ar % 16, ar // 16] = li
        return np.tile(w, (8, 1))

    in_maps = []
    for c in range(NCORES):
        _, _, pair_lists, single_lists, _ = per_core[c]
        # The ucode contract: num_idxs_reg == count of NON-NEGATIVE idx in
        # the chunk's slice (violations fault the exec unit). Real entries,
        # then 0-pads up to each chunk's register count, then -1.
        cnts = np.empty((NSUB, ncls), np.int32)
        lips, liss = [], []
        for s in range(NSUB):
            lip = np.full(NP_, -1, np.int16)
            lip[: len(pair_lists[s])] = pair_lists[s]
            lis_ = np.full(NS_, -1, np.int16)
            lis_[: len(single_lists[s])] = single_lists[s]
            for j, (cls, o, sz) in enumerate(plan):
                li = lip if cls == "p" else lis_
                n = len(pair_lists[s]) if cls == "p" else len(single_lists[s])
                e = max(min(n - o, sz), min(16, sz))
                lo, hi = max(n, o), o + e
                if hi > lo:
                    li[lo:hi] = 0
                cnts[s, j] = e
            lips.append(lip)
            liss.append(lis_)
        in_maps.append({
            "shard16": np.ascontiguousarray(table16[c * SHARD : (c + 1) * SHARD]),
            "idxp": np.ascontiguousarray(np.concatenate([wrap(x) for x in lips], axis=1)),
            "idxs": np.ascontiguousarray(np.concatenate([wrap(x) for x in liss], axis=1)),
            "cnt": cnts.reshape(1, NSUB * ncls),
        })

    const_full = [
        all(
            min(len(pc[2][s] if cls == "p" else pc[3][s]) - o, sz) == sz
            for pc in per_core
            for s in range(NSUB)
        )
        for (cls, o, sz) in plan
    ]
    nc = _build_program(NP_, NS_, plan, const_full)
    LAST_RUN = run_bass_kernel_spmd(nc, in_maps, list(range(NCORES)))
    res = LAST_RUN.results

    emb = np.empty((total, VEC), np.float16)
    for c in range(NCORES):
        pos, inv, pair_lists, single_lists, maps = per_core[c]
        op = np.asarray(res[c]["out_p"]).view(np.float16).reshape(P, NSUB, CP_, 2 * VEC)
        os_ = np.asarray(res[c]["out_s"]).view(np.float16).reshape(P, NSUB, CS_, VEC)
        emb_u_parts = []
        for s in range(NSUB):
            is_pair, pair_row, single_slot = maps[s]
            n = len(is_pair)
            if n == 0:
                continue
            # pair slot k landed at [k%128, k//128, :] as 2*VEC elems
            prows = op[:, s].transpose(1, 0, 2).reshape(NP_ * 2, VEC)
            srows = os_[:, s].transpose(1, 0, 2).reshape(NS_, VEC)
            eu = np.empty((n, VEC), np.float16)
            eu[is_pair] = prows[pair_row[is_pair]]
            eu[~is_pair] = srows[single_slot[~is_pair]]
            emb_u_parts.append(eu)
        emb_u = np.concatenate(emb_u_parts, axis=0)
        emb[pos] = emb_u[inv]
    return emb.astype(np.float32)


def kernel(table, row_offsets, value_tensors, nnz_array=None, output_shape=None):
    table = np.ascontiguousarray(np.asarray(table, dtype=np.float32))
    assert table.shape == (VOCAB, VEC)
    v = np.asarray(value_tensors).astype(np.int64).ravel()
    total = v.shape[0]

    emb = _gather_on_device(table, v)

    n_rows = BATCH * SLOTS
    ro = np.asarray(row_offsets).astype(np.int64).ravel()
    if total == n_rows and np.array_equal(ro, np.arange(total + 1)):
        return emb.reshape(BATCH, SLOTS, VEC)
    # General CSR fallback (never hit with the reference's arange offsets):
    # sum-combine values per segment on the host.
    seg = np.searchsorted(ro, np.arange(total), side="right") - 1
    combined = np.zeros((n_rows, VEC), np.float32)
    np.add.at(combined, seg, emb)
    return combined.reshape(BATCH, SLOTS, VEC)
